# revision 11
# baseline (speedup 1.0000x reference)
"""Windowed multi-head attention (PointPillar what2keep) on 8 TRN2 NeuronCores.

Data-parallel over the 256 windows (32 windows/core). Per window:
  qkv proj (fp32r matmuls) -> simT = k@q^T per head (bf16, row-packed PE)
  -> exp (ACT) -> * exp(bias) (DVE/GPSIMD, host-precomputed) -> row sums +
  av^T (col-packed PE with ones / v) -> reciprocal + normalize (DVE)
  -> out proj (fp32r) -> DMA out.
Softmax needs no max-subtraction: weights are randn*0.02 so |logits| << 1.
"""

import sys

sys.path.insert(0, "/opt/trn_rl_repo")

import ml_dtypes
import numpy as np

import concourse.bass as bass
import concourse.bacc as bacc
import concourse.tile as tile
from concourse import mybir
from concourse._compat import with_exitstack
from concourse.bass_utils import run_bass_kernel_spmd

AGENT, WIN, GRID, D, H, DH = 6, 7, 16, 256, 8, 32
N = AGENT * WIN * WIN          # 294 tokens per window
NWIN = GRID * GRID             # 256 windows
NCORES = 8
WPC = NWIN // NCORES           # 32 windows per core
NT = 98                        # token tile; 294 = 3 * 98

F32 = mybir.dt.float32
F32R = mybir.dt.float32r
BF16 = mybir.dt.bfloat16


@with_exitstack
def _emit(ctx, tc, xt, ebt, wqkv, wout, out):
    nc = tc.nc
    dma = nc.default_dma_engine

    const = ctx.enter_context(tc.sbuf_pool(name="const", bufs=1))
    wqkv_sb = const.tile([128, 2, 3 * D], F32R, name="wqkv_sb")   # [c_loc, cblk, col]
    wout_sb = const.tile([128, 2, D], F32R, name="wout_sb")       # [d_loc, g, dout]
    ebt_sb = const.tile([NT, 4, 3, 2, N], BF16, name="ebt_sb")   # [jl, pair, jt, hl, i]
    ones_sb = const.tile([NT, DH], BF16, name="ones_sb")
    # staging: matmul operands must be produced by compute engines, not DMA
    # (DMA dep edges overflow the LDWEIGHTS sync-wait budget in walrus)
    wqkv_st = const.tile([128, 2, 3 * D], F32R, name="wqkv_st")
    wout_st = const.tile([128, 2, D], F32R, name="wout_st")

    for cb in range(2):
        dma.dma_start(out=wqkv_st[:, cb, :], in_=wqkv[cb])
        dma.dma_start(out=wout_st[:, cb, :], in_=wout[cb])
    for pair in range(4):
        for jt in range(3):
            dma.dma_start(out=ebt_sb[:, pair, jt, :, :], in_=ebt[pair, jt])
    nc.vector.tensor_copy(out=wqkv_sb[:, :, :], in_=wqkv_st[:, :, :])
    nc.vector.tensor_copy(out=wout_sb[:, :, :], in_=wout_st[:, :, :])
    nc.vector.memset(ones_sb[:, :], 1.0)

    xp = ctx.enter_context(tc.sbuf_pool(name="xp", bufs=2))
    xsp = ctx.enter_context(tc.sbuf_pool(name="xsp", bufs=2))
    qkp = ctx.enter_context(tc.sbuf_pool(name="qkp", bufs=2))
    vp = ctx.enter_context(tc.sbuf_pool(name="vp", bufs=2))
    ep = ctx.enter_context(tc.sbuf_pool(name="ep", bufs=3))
    app = ctx.enter_context(tc.sbuf_pool(name="app", bufs=3))
    rp = ctx.enter_context(tc.sbuf_pool(name="rp", bufs=2))
    avp = ctx.enter_context(tc.sbuf_pool(name="avp", bufs=2))
    op = ctx.enter_context(tc.sbuf_pool(name="op", bufs=2))

    ps_pair = ctx.enter_context(tc.psum_pool(name="ps_pair", bufs=2))  # 2 banks/tile
    ps_s = ctx.enter_context(tc.psum_pool(name="ps_s", bufs=2))        # 1 bank/tile
    ps_av = ctx.enter_context(tc.psum_pool(name="ps_av", bufs=2))      # 1 bank/tile

    for w in range(WPC):
        xst = xsp.tile([128, 2, N], F32R, name=f"xs_{w}", tag="xs")
        for cb in range(2):
            dma.dma_start(out=xst[:, cb, :], in_=xt[w, cb])
        xtile = xp.tile([128, 2, N], F32R, name=f"x_{w}", tag="x")
        nc.vector.tensor_copy(out=xtile[:, :, :], in_=xst[:, :, :])

        # ---- q^T / k^T projection: (128 dims, 294 toks) per half ----
        q_ps = ps_pair.tile([128, 2, 512], F32, name=f"qps_{w}", tag="pair")
        k_ps = ps_pair.tile([128, 2, 512], F32, name=f"kps_{w}", tag="pair")
        for t in range(4):  # wqkv col chunks: q0 q1 k0 k1
            dst = q_ps if t < 2 else k_ps
            for cb in range(2):
                nc.tensor.matmul(
                    dst[:, t % 2, :N],
                    wqkv_sb[:, cb, 128 * t:128 * (t + 1)],
                    xtile[:, cb, :],
                    start=(cb == 0),
                    stop=(cb == 1),
                )
        qT = qkp.tile([128, 2, N], BF16, name=f"qT_{w}", tag="qT")  # [32h_loc+d, g, i]
        kT = qkp.tile([128, 2, N], BF16, name=f"kT_{w}", tag="kT")
        nc.vector.tensor_copy(out=qT[:, :, :], in_=q_ps[:, :, :N])
        nc.vector.tensor_copy(out=kT[:, :, :], in_=k_ps[:, :, :N])

        # ---- v: (98 toks, 256 dims) per token tile ----
        v_sb = vp.tile([NT, 3, D], BF16, name=f"v_{w}", tag="v")  # [jl, jt, d]
        for jt in range(3):
            v_ps = ps_s.tile([128, 512], F32, name=f"vps_{w}_{jt}", tag="s")
            for cb in range(2):
                nc.tensor.matmul(
                    v_ps[:NT, :D],
                    xtile[:, cb, 98 * jt:98 * jt + 98],
                    wqkv_sb[:, cb, 2 * D:3 * D],
                    start=(cb == 0),
                    stop=(cb == 1),
                )
            nc.vector.tensor_copy(out=v_sb[:, jt, :], in_=v_ps[:NT, :D])

        # ---- attention per head-group g (heads 4g..4g+3) ----
        rt = rp.tile([128, 2, N], F32, name=f"rt_{w}", tag="rt")
        avn = avp.tile([128, 2, N], F32R, name=f"avn_{w}", tag="avn")
        for g in range(2):
            s_ps = ps_s.tile([128, 512], F32, name=f"sps_{w}_{g}", tag="s")
            av_ps = ps_av.tile([128, 512], F32, name=f"avps_{w}_{g}", tag="av")
            for jt in range(3):
                pairs = [
                    ps_pair.tile([128, 2, 512], F32, name=f"sim_{w}_{g}_{jt}_{p}", tag="pair")
                    for p in range(2)
                ]
                # simT[j, i] = k[j] . q[i] for 4 heads, row-packed on the PE
                for k in range(4):
                    nc.tensor.matmul(
                        pairs[k // 2][:NT, k % 2, :N],
                        kT[32 * k:32 * k + 32, g, 98 * jt:98 * jt + 98],
                        qT[32 * k:32 * k + 32, g, :],
                        start=True,
                        stop=True,
                        tile_position=(32 * k, 0),
                    )
                exps = ep.tile([NT, 2, 2, N], BF16, name=f"exps_{w}_{g}_{jt}", tag="exps")
                for p in range(2):
                    nc.scalar.activation(
                        out=exps[:, p, :, :],
                        in_=pairs[p][:NT, :, :N],
                        func=mybir.ActivationFunctionType.Exp,
                    )
                attn = app.tile([NT, 2, 2, N], BF16, name=f"attn_{w}_{g}_{jt}", tag="attn")
                for p in range(2):
                    eng = nc.gpsimd if (g * 6 + jt * 2 + p) % 3 == 2 else nc.vector
                    eng.tensor_tensor(
                        attn[:, p, :, :],
                        exps[:, p, :, :],
                        ebt_sb[:, 2 * g + p, jt, :, :],
                        mybir.AluOpType.mult,
                    )
                # row sums (ones) + av^T (v), col-packed, accumulated over jt
                for k in range(4):
                    a_sl = attn[:, k // 2, k % 2, :]
                    nc.tensor.matmul(
                        s_ps[32 * k:32 * k + 32, :N],
                        ones_sb[:, :],
                        a_sl,
                        start=(jt == 0),
                        stop=(jt == 2),
                        tile_position=(0, 32 * k),
                    )
                    h = 4 * g + k
                    nc.tensor.matmul(
                        av_ps[32 * k:32 * k + 32, :N],
                        v_sb[:, jt, DH * h:DH * h + DH],
                        a_sl,
                        start=(jt == 0),
                        stop=(jt == 2),
                        tile_position=(0, 32 * k),
                    )
            nc.vector.reciprocal(out=rt[:, g, :], in_=s_ps[:, :N])
            nc.vector.tensor_tensor(
                avn[:, g, :], av_ps[:, :N], rt[:, g, :], mybir.AluOpType.mult
            )

        # ---- output projection ----
        o_sb = op.tile([NT, 3, D], F32, name=f"o_{w}", tag="o")  # [il, it, dout]
        for it in range(3):
            o_ps = ps_s.tile([128, 512], F32, name=f"ops_{w}_{it}", tag="s")
            for g in range(2):
                nc.tensor.matmul(
                    o_ps[:NT, :D],
                    avn[:, g, 98 * it:98 * it + 98],
                    wout_sb[:, g, :],
                    start=(g == 0),
                    stop=(g == 1),
                )
            nc.vector.tensor_copy(out=o_sb[:, it, :], in_=o_ps[:NT, :D])
            nc.gpsimd.dma_start(out=out[w, 98 * it:98 * it + 98, :], in_=o_sb[:, it, :])


def _build():
    nc = bacc.Bacc()
    xt = nc.declare_dram_parameter("xt", [WPC, 2, 128, N], F32R, isOutput=False)
    ebt = nc.declare_dram_parameter("ebt", [4, 3, NT, 2, N], BF16, isOutput=False)
    wqkv = nc.declare_dram_parameter("wqkv", [2, 128, 3 * D], F32R, isOutput=False)
    wout = nc.declare_dram_parameter("wout", [2, 128, D], F32R, isOutput=False)
    out = nc.declare_dram_parameter("out", [WPC, N, D], F32, isOutput=True)
    with tile.TileContext(nc) as tc:
        _emit(tc, xt, ebt, wqkv, wout, out)
    nc.finalize()
    return nc


_NC = None
_last_in_maps = None


def kernel(x, w_qkv, w_out, bias_table, rel_index):
    global _NC
    if _NC is None:
        _NC = _build()

    x = np.asarray(x, np.float32)
    # (b,l,X,Y,w1,w2,d) -> (X,Y,d,l,w1,w2) -> (win, d, tok)
    xT = np.ascontiguousarray(
        x[0].transpose(1, 2, 5, 0, 3, 4).reshape(NWIN, D, N)
    )

    wq = np.asarray(w_qkv, np.float32).copy()
    wq[:, :D] *= DH ** -0.5                      # fold q scale
    wq = np.ascontiguousarray(wq.reshape(2, 128, 3 * D))
    wo = np.ascontiguousarray(np.asarray(w_out, np.float32).reshape(2, 128, D))

    # exp(bias)[h, j, i], laid out [pair, jt, jl, hl, i] with h = 2*pair + hl
    eb = np.exp(np.asarray(bias_table, np.float32)[np.asarray(rel_index)])  # [i,j,h]
    eb = eb.transpose(2, 1, 0)                   # [h, j, i]
    eb = eb.reshape(4, 2, 3, NT, N).transpose(0, 2, 3, 1, 4)
    eb = np.ascontiguousarray(eb).astype(ml_dtypes.bfloat16)

    in_maps = [
        {
            "xt": np.ascontiguousarray(
                xT[c * WPC:(c + 1) * WPC].reshape(WPC, 2, 128, N)
            ),
            "ebt": eb,
            "wqkv": wq,
            "wout": wo,
        }
        for c in range(NCORES)
    ]
    global _last_in_maps
    _last_in_maps = in_maps
    res = run_bass_kernel_spmd(_NC, in_maps, list(range(NCORES))).results
    full = np.concatenate(
        [np.asarray(res[c]["out"], np.float32) for c in range(NCORES)], axis=0
    )  # (256 win, 294 tok, 256 d)
    y = full.reshape(1, GRID, GRID, AGENT, WIN, WIN, D).transpose(0, 3, 1, 2, 4, 5, 6)
    return np.ascontiguousarray(y)


# revision 12
# speedup vs baseline: 1.1338x; 1.1338x over previous
"""Windowed multi-head attention (PointPillar what2keep) on 8 TRN2 NeuronCores.

Data-parallel over the 256 windows (32 windows/core). Per window:
  qkv proj (bf16 matmuls) -> simT = k@q^T per head (bf16, row-packed PE)
  -> exp (ACT) -> * exp(bias) (DVE/GPSIMD 2x-mode TT, host-precomputed)
  -> row sums + av^T (col-packed PE with ones / v) -> reciprocal_approx_fast
  + normalize (DVE) -> out proj (bf16) -> DMA out (sync engine).
Softmax needs no max-subtraction: weights are randn*0.02 so |logits| << 1.
"""

import sys

sys.path.insert(0, "/opt/trn_rl_repo")

import ml_dtypes
import numpy as np

import concourse.bacc as bacc
import concourse.tile as tile
from concourse import mybir
from concourse._compat import with_exitstack
from concourse.bass_utils import run_bass_kernel_spmd

AGENT, WIN, GRID, D, H, DH = 6, 7, 16, 256, 8, 32
N = AGENT * WIN * WIN          # 294 tokens per window
NWIN = GRID * GRID             # 256 windows
NCORES = 8
WPC = NWIN // NCORES           # 32 windows per core
NT = 98                        # token tile; 294 = 3 * 98

F32 = mybir.dt.float32
BF16 = mybir.dt.bfloat16


@with_exitstack
def _emit(ctx, tc, xt, ebt, wqkv, wout, out):
    nc = tc.nc
    dma = nc.default_dma_engine  # sync engine (hardware DGE), otherwise idle

    const = ctx.enter_context(tc.sbuf_pool(name="const", bufs=1))
    wqkv_sb = const.tile([128, 2, 3 * D], BF16, name="wqkv_sb")  # [c_loc, cb, col]
    wout_sb = const.tile([128, 2, D], BF16, name="wout_sb")      # [d_loc, g, dout]
    ebt_sb = const.tile([NT, 4, 3, 2 * N], BF16, name="ebt_sb")  # [jl, pair, jt, hl*i]
    ones_sb = const.tile([NT, DH], BF16, name="ones_sb")

    for cb in range(2):
        dma.dma_start(out=wqkv_sb[:, cb, :], in_=wqkv[cb])
        dma.dma_start(out=wout_sb[:, cb, :], in_=wout[cb])
    for pair in range(4):
        for jt in range(3):
            dma.dma_start(out=ebt_sb[:, pair, jt, :], in_=ebt[pair, jt])
    nc.vector.memset(ones_sb[:, :], 1.0)

    xp = ctx.enter_context(tc.sbuf_pool(name="xp", bufs=2))
    qkp = ctx.enter_context(tc.sbuf_pool(name="qkp", bufs=2))
    vp = ctx.enter_context(tc.sbuf_pool(name="vp", bufs=2))
    ep = ctx.enter_context(tc.sbuf_pool(name="ep", bufs=3))
    app = ctx.enter_context(tc.sbuf_pool(name="app", bufs=3))
    rp = ctx.enter_context(tc.sbuf_pool(name="rp", bufs=2))
    avp = ctx.enter_context(tc.sbuf_pool(name="avp", bufs=2))
    op = ctx.enter_context(tc.sbuf_pool(name="op", bufs=2))

    ps_pair = ctx.enter_context(tc.psum_pool(name="ps_pair", bufs=2))  # 2 banks/tile
    ps_s = ctx.enter_context(tc.psum_pool(name="ps_s", bufs=2))        # 1 bank/tile
    ps_av = ctx.enter_context(tc.psum_pool(name="ps_av", bufs=2))      # 1 bank/tile

    for w in range(WPC):
        xtile = xp.tile([128, 2, N], BF16, name=f"x_{w}", tag="x")
        dma.dma_start(out=xtile[:, :, :], in_=xt[w])

        # ---- q^T / k^T projection: (128 dims, 294 toks) per half ----
        q_ps = ps_pair.tile([128, 2, 512], F32, name=f"qps_{w}", tag="pair")
        k_ps = ps_pair.tile([128, 2, 512], F32, name=f"kps_{w}", tag="pair")
        for t in range(4):  # wqkv col chunks: q0 q1 k0 k1
            dst = q_ps if t < 2 else k_ps
            for cb in range(2):
                nc.tensor.matmul(
                    dst[:, t % 2, :N],
                    wqkv_sb[:, cb, 128 * t:128 * (t + 1)],
                    xtile[:, cb, :],
                    start=(cb == 0),
                    stop=(cb == 1),
                )
        qT = qkp.tile([128, 2, N], BF16, name=f"qT_{w}", tag="qT")  # [32h_loc+d, g, i]
        kT = qkp.tile([128, 2, N], BF16, name=f"kT_{w}", tag="kT")
        nc.scalar.copy(out=qT[:, :, :], in_=q_ps[:, :, :N])
        nc.vector.tensor_copy(out=kT[:, :, :], in_=k_ps[:, :, :N])

        # ---- v: (98 toks, 256 dims) per token tile ----
        v_sb = vp.tile([NT, 3, D], BF16, name=f"v_{w}", tag="v")  # [jl, jt, d]
        for jt in range(3):
            v_ps = ps_s.tile([128, 512], F32, name=f"vps_{w}_{jt}", tag="s")
            for cb in range(2):
                nc.tensor.matmul(
                    v_ps[:NT, :D],
                    xtile[:, cb, 98 * jt:98 * jt + 98],
                    wqkv_sb[:, cb, 2 * D:3 * D],
                    start=(cb == 0),
                    stop=(cb == 1),
                )
            nc.vector.tensor_copy(out=v_sb[:, jt, :], in_=v_ps[:NT, :D])

        # ---- attention per head-group g (heads 4g..4g+3) ----
        rt = rp.tile([128, 2, N], F32, name=f"rt_{w}", tag="rt")
        avn = avp.tile([128, 2, N], BF16, name=f"avn_{w}", tag="avn")
        for g in range(2):
            s_ps = ps_s.tile([128, 512], F32, name=f"sps_{w}_{g}", tag="s")
            av_ps = ps_av.tile([128, 512], F32, name=f"avps_{w}_{g}", tag="av")
            for jt in range(3):
                pairs = [
                    ps_pair.tile([128, 2, 512], F32, name=f"sim_{w}_{g}_{jt}_{p}", tag="pair")
                    for p in range(2)
                ]
                # simT[j, i] = k[j] . q[i] for 4 heads, row-packed on the PE
                for k in range(4):
                    nc.tensor.matmul(
                        pairs[k // 2][:NT, k % 2, :N],
                        kT[32 * k:32 * k + 32, g, 98 * jt:98 * jt + 98],
                        qT[32 * k:32 * k + 32, g, :],
                        start=True,
                        stop=True,
                        tile_position=(32 * k, 0),
                    )
                exps = ep.tile([NT, 2, 2 * N], BF16, name=f"exps_{w}_{g}_{jt}", tag="exps")
                for p in range(2):
                    nc.scalar.activation(
                        out=exps[:, p, :],
                        in_=pairs[p][:NT, :, :N],
                        func=mybir.ActivationFunctionType.Exp,
                    )
                attn = app.tile([NT, 2, 2 * N], BF16, name=f"attn_{w}_{g}_{jt}", tag="attn")
                for p in range(2):
                    m = g * 6 + jt * 2 + p
                    eng = nc.gpsimd if m % 4 == 3 else nc.vector
                    eng.tensor_tensor(
                        attn[:, p, :],
                        exps[:, p, :],
                        ebt_sb[:, 2 * g + p, jt, :],
                        mybir.AluOpType.mult,
                    )
                # row sums (ones) + av^T (v), col-packed, accumulated over jt
                for k in range(4):
                    a_sl = attn[:, k // 2, N * (k % 2):N * (k % 2) + N]
                    nc.tensor.matmul(
                        s_ps[32 * k:32 * k + 32, :N],
                        ones_sb[:, :],
                        a_sl,
                        start=(jt == 0),
                        stop=(jt == 2),
                        tile_position=(0, 32 * k),
                    )
                    h = 4 * g + k
                    nc.tensor.matmul(
                        av_ps[32 * k:32 * k + 32, :N],
                        v_sb[:, jt, DH * h:DH * h + DH],
                        a_sl,
                        start=(jt == 0),
                        stop=(jt == 2),
                        tile_position=(0, 32 * k),
                    )
            nc.vector.reciprocal_approx_fast(out=rt[:, g, :], in_=s_ps[:, :N])
            nc.vector.tensor_tensor(
                avn[:, g, :], av_ps[:, :N], rt[:, g, :], mybir.AluOpType.mult
            )

        # ---- output projection ----
        o_sb = op.tile([NT, 3, D], F32, name=f"o_{w}", tag="o")  # [il, it, dout]
        for it in range(3):
            o_ps = ps_s.tile([128, 512], F32, name=f"ops_{w}_{it}", tag="s")
            for g in range(2):
                nc.tensor.matmul(
                    o_ps[:NT, :D],
                    avn[:, g, 98 * it:98 * it + 98],
                    wout_sb[:, g, :],
                    start=(g == 0),
                    stop=(g == 1),
                )
            nc.vector.tensor_copy(out=o_sb[:, it, :], in_=o_ps[:NT, :D])
            dma.dma_start(out=out[w, 98 * it:98 * it + 98, :], in_=o_sb[:, it, :])


def _build():
    nc = bacc.Bacc()
    xt = nc.declare_dram_parameter("xt", [WPC, 128, 2, N], BF16, isOutput=False)
    ebt = nc.declare_dram_parameter("ebt", [4, 3, NT, 2 * N], BF16, isOutput=False)
    wqkv = nc.declare_dram_parameter("wqkv", [2, 128, 3 * D], BF16, isOutput=False)
    wout = nc.declare_dram_parameter("wout", [2, 128, D], BF16, isOutput=False)
    out = nc.declare_dram_parameter("out", [WPC, N, D], F32, isOutput=True)
    with tile.TileContext(nc) as tc:
        _emit(tc, xt, ebt, wqkv, wout, out)
    nc.finalize()
    return nc


_NC = None
_last_in_maps = None


def kernel(x, w_qkv, w_out, bias_table, rel_index):
    global _NC
    if _NC is None:
        _NC = _build()

    x = np.asarray(x, np.float32)
    # (b,l,X,Y,w1,w2,d) -> (X,Y,d,l,w1,w2) -> (win, d, tok)
    xT = np.ascontiguousarray(
        x[0].transpose(1, 2, 5, 0, 3, 4).reshape(NWIN, D, N)
    )

    wq = np.asarray(w_qkv, np.float32).copy()
    wq[:, :D] *= DH ** -0.5                      # fold q scale
    wq = np.ascontiguousarray(wq.reshape(2, 128, 3 * D)).astype(ml_dtypes.bfloat16)
    wo = np.ascontiguousarray(
        np.asarray(w_out, np.float32).reshape(2, 128, D)
    ).astype(ml_dtypes.bfloat16)

    # exp(bias)[h, j, i], laid out [pair, jt, jl, hl*i] with h = 2*pair + hl
    eb = np.exp(np.asarray(bias_table, np.float32)[np.asarray(rel_index)])  # [i,j,h]
    eb = eb.transpose(2, 1, 0)                   # [h, j, i]
    eb = eb.reshape(4, 2, 3, NT, N).transpose(0, 2, 3, 1, 4).reshape(4, 3, NT, 2 * N)
    eb = np.ascontiguousarray(eb).astype(ml_dtypes.bfloat16)

    in_maps = [
        {
            "xt": np.ascontiguousarray(
                xT[c * WPC:(c + 1) * WPC]
                .reshape(WPC, 2, 128, N)
                .transpose(0, 2, 1, 3)
            ).astype(ml_dtypes.bfloat16),
            "ebt": eb,
            "wqkv": wq,
            "wout": wo,
        }
        for c in range(NCORES)
    ]
    global _last_in_maps
    _last_in_maps = in_maps
    res = run_bass_kernel_spmd(_NC, in_maps, list(range(NCORES))).results
    full = np.concatenate(
        [np.asarray(res[c]["out"], np.float32) for c in range(NCORES)], axis=0
    )  # (256 win, 294 tok, 256 d)
    y = full.reshape(1, GRID, GRID, AGENT, WIN, WIN, D).transpose(0, 3, 1, 2, 4, 5, 6)
    return np.ascontiguousarray(y)
